# revision 25
# baseline (speedup 1.0000x reference)
"""nn_ContrastiveLoss Trainium2 kernel (8 NeuronCores, data-parallel over batch).

Contract: kernel(embeddings=[64,1024,128] f32, labels=[64,1024] int64) -> f32 scalar.

Sharding: batch dim B=64 split as 8 samples per core. Host packs each sample's
rows by label with the SMALLER group first (the "A side", <= 512 rows always,
zero-padded to 512) and the larger group second ("B side", zero-padded to
PADB), one uniform shape for all cores, cast to fp16 (halves HBM traffic).

Device pipeline per sample (measured-rate balanced):
  - DMA *xbar transpose* load: [rows,128] DRAM -> [128d, rows] SBUF fp16
    directly (no PE transposes, no diag build, no PSUM->SBUF copies)
  - squares on DVE (fp16 tensor_tensor, 2x mode)
  - row norms^2 via PE ones-matmuls (contraction over partitions), then tiny
    col-to-partition / rectangle DMA unflattens into a [128,20] layout where
    sqrt (ACT) and reciprocal (DVE) run on all lanes
  - B-side normalize: GpSimd partition_broadcast of rinvB (its only per-sample
    op type - GpSimd ucode swaps cost ~5us) + DVE fp16 multiply
  - A-side normalize is folded into the hinge via
      relu(rinvA_i*G' - th) = rinvA_i * relu(G' - th*rA_i):
    per-partition bias -th*rA_i in the hinge op, rinvA_i applied later to the
    accumulated per-row sums (one tiny batched multiply)
  - PE sim matmuls fp16 raw-A x normalized-B -> PSUM fp32
  - fused hinge+row-sum in ONE pass per chunk: ACT Relu(bias)+accum_out and
    DVE scalar_tensor_tensor (G+bias) max 0 with accum_out, exact (no
    correction term), split across both engines
  - epilogue: slots * rinvA, reduce, GpSimd cross-partition all-reduce,
    * host-provided valid/max(n_neg,1), DMA [1,8] losses
Host sums the 64 per-sample losses, divides by the label-derived count.
"""

import sys

if "/opt/trn_rl_repo" not in sys.path:
    sys.path.insert(0, "/opt/trn_rl_repo")

from contextlib import ExitStack

import numpy as np

import concourse.bass as bass
import concourse.bacc as bacc
import concourse.mybir as mybir
import concourse.tile as tile
from concourse import bass_isa, bass_utils

F32 = mybir.dt.float32
F16 = mybir.dt.float16
AF = mybir.ActivationFunctionType
ALU = mybir.AluOpType

P = 128
D = 128
N = 1024
B = 64
NCORES = 8
BPC = B // NCORES
PADA = 512          # lhsT side (min(npos,nneg) <= 512 always)
NA = PADA // P      # 4 A-chunks
THRESH = 0.5 - 0.35
EPS2 = 1e-6

# hinge engine per (sample, [wide0, wide1, tail]): wide = [128, 2*512] over
# two chunks' 512-wide sim parts; tail = [128, 4*64] over all four chunks'
# B-tail parts. Split ACT/DVE by measured rates (~9 ACT wides / 7 DVE wides).
_HINGE_PAT = [
    ["act", "dve", "dve"],
    ["act", "act", "dve"],
    ["act", "dve", "act"],
    ["act", "dve", "dve"],
] * (BPC // 4)


def _kernel_body(ctx, tc, emb_ap, winv_ap, out_ap, bpc, padb):
    nc = tc.nc
    rows = PADA + padb
    nbp = padb // 16          # B-side partitions in [*,16] nat layout (36)

    const_pool = ctx.enter_context(tc.tile_pool(name="const", bufs=1))
    xtpool = ctx.enter_context(tc.tile_pool(name="xt", bufs=3))
    sqpool = ctx.enter_context(tc.tile_pool(name="sq", bufs=2))
    nrmpool = ctx.enter_context(tc.tile_pool(name="nrm", bufs=2))
    ebpool = ctx.enter_context(tc.tile_pool(name="eb", bufs=2))
    acc_pool = ctx.enter_context(tc.tile_pool(name="acc", bufs=1))
    nsq_psum = ctx.enter_context(tc.tile_pool(name="nsqps", bufs=1, space="PSUM"))
    sim_psum = ctx.enter_context(tc.tile_pool(name="simps", bufs=2, space="PSUM"))
    simb_psum = ctx.enter_context(tc.tile_pool(name="simbps", bufs=1, space="PSUM"))

    eps2 = const_pool.tile([P, 1], F32)
    nc.gpsimd.memset(eps2[:], EPS2)
    zz16 = const_pool.tile([P, 1], F16)
    nc.gpsimd.memset(zz16[:], 0.0)
    onesw = const_pool.tile([P, 1], F16)
    nc.gpsimd.memset(onesw[:], 1.0)

    warm = const_pool.tile([P, 1], F32)
    nc.scalar.activation(warm[:], eps2[:], AF.Abs_reciprocal_sqrt, bias=eps2[:])

    # Pre-warm the xbar-transpose and partition_broadcast mechanisms: their
    # first use signals completion early (cold-start race seen on sample 0).
    wxb = const_pool.tile([P, 16], F16)
    nc.sync.dma_start_transpose(wxb[:], emb_ap[0][0:16, :])
    wbc = const_pool.tile([P, 16], F16)
    nc.gpsimd.partition_broadcast(wbc[:], wxb[0:1, :])
    wsq = const_pool.tile([P, 16], F16)
    nc.vector.tensor_mul(wsq[:], wxb[:], wbc[:])

    winv_sb = const_pool.tile([1, bpc], F32)
    nc.sync.dma_start(winv_sb[:], winv_ap[:])

    slots = acc_pool.tile([P, bpc, 3], F32)
    nc.gpsimd.memset(slots[:], 0.0)
    neg_thr = const_pool.tile([P, 1], F32)
    nc.gpsimd.memset(neg_thr[:], -THRESH)

    hsc_a = const_pool.tile([P, 1024], F16)
    hsc_d = const_pool.tile([P, 1024], F16)

    xt_t, eb_t = {}, {}

    def emit_load(b):
        xt = xtpool.tile([P, rows], F16, tag="xt")
        xt_t[b] = xt
        nc.sync.dma_start_transpose(xt[:], emb_ap[b])

    def emit_norm(b):
        xt = xt_t[b]
        esq = sqpool.tile([P, rows], F16, tag="esq")
        nc.vector.tensor_mul(esq[:], xt[:], xt[:])

        # norms^2 via ones-matmuls into one flat PSUM tile; ONE ACT
        # Abs_reciprocal_sqrt = PSUM exit + rsqrt fused, single rfl writer
        # (keeps the GpSimd broadcast's semaphore count down)
        ps = nsq_psum.tile([1, rows], F32, tag="nsq")
        for c0 in range(0, rows, 512):
            w = min(512, rows - c0)
            nc.tensor.matmul(ps[0:1, c0:c0 + w], lhsT=onesw[:],
                             rhs=esq[:, c0:c0 + w], start=True, stop=True)
        rfl = nrmpool.tile([1, rows], F16, tag="rfl")
        nc.scalar.activation(rfl[:], ps[:], AF.Abs_reciprocal_sqrt,
                             bias=eps2[0:1, :])
        rbc = nrmpool.tile([P, rows], F16, tag="rbc")
        nc.gpsimd.partition_broadcast(rbc[:], rfl[0:1, :])
        eh = ebpool.tile([P, rows], F16, tag="eh")
        eb_t[b] = eh
        nc.vector.tensor_mul(eh[:], xt[:], rbc[:])

    tailw = padb - 512

    def hinge(which, src, shape, slot):
        if which == "act":
            nc.scalar.activation(hsc_a[:, 0:shape], src, AF.Relu,
                                 bias=neg_thr[:], accum_out=slot)
        else:
            nc.vector.scalar_tensor_tensor(
                hsc_d[:, 0:shape], src, THRESH,
                zz16[:].broadcast_to(src.shape),
                ALU.subtract, ALU.max, accum_out=slot)

    def emit_sims(b):
        eh = eb_t[b]
        # B-tail parts of all four chunks share one 1-bank tile, hinged once
        # (emitted before the last wide hinge so the next sample's tail
        # matmuls aren't gated on this sample's final hinge)
        sim_b = simb_psum.tile([P, NA, tailw], F32, tag="simb")
        sims_a = []
        for half in range(2):
            sim_a = sim_psum.tile([P, 2, 512], F32, tag="sima")
            sims_a.append(sim_a)
            for q in range(2):
                mt = half * 2 + q
                lhs = eh[:, mt * P:(mt + 1) * P]
                nc.tensor.matmul(sim_a[:, q, :], lhsT=lhs,
                                 rhs=eh[:, PADA:PADA + 512],
                                 start=True, stop=True)
                nc.tensor.matmul(sim_b[:, mt, :], lhsT=lhs,
                                 rhs=eh[:, PADA + 512:rows],
                                 start=True, stop=True)
            if half == 0:
                hinge(_HINGE_PAT[b][0], sim_a[:], 1024,
                      slots[:, b, 0:1])
        hinge(_HINGE_PAT[b][2], sim_b[:], NA * tailw, slots[:, b, 2:3])
        hinge(_HINGE_PAT[b][1], sims_a[1][:], 1024, slots[:, b, 1:2])
        del xt_t[b], eb_t[b]

    emit_load(0)
    emit_load(1)
    emit_norm(0)
    for b in range(bpc):
        if b + 2 < bpc:
            emit_load(b + 2)
        if b + 1 < bpc:
            emit_norm(b + 1)
        emit_sims(b)

    red = acc_pool.tile([P, bpc], F32)
    nc.vector.tensor_reduce(red[:], slots[:], axis=mybir.AxisListType.X,
                            op=ALU.add)
    redr = acc_pool.tile([P, bpc], F32)
    nc.gpsimd.partition_all_reduce(redr[:], red[:], channels=P,
                                   reduce_op=bass_isa.ReduceOp.add)
    loss = acc_pool.tile([1, bpc], F32)
    nc.vector.tensor_mul(loss[:], redr[0:1, :], winv_sb[:])
    nc.sync.dma_start(out_ap[0:1, :], loss[0:1, :])


_NC_CACHE = {}


def _build(padb):
    key = (BPC, NCORES, padb)
    if key in _NC_CACHE:
        return _NC_CACHE[key]
    nc = bacc.Bacc("TRN2", target_bir_lowering=False, debug=False,
                   num_devices=NCORES)
    emb = nc.dram_tensor("emb", [BPC, PADA + padb, D], F16,
                         kind="ExternalInput")
    winv = nc.dram_tensor("winv", [1, BPC], F32, kind="ExternalInput")
    out = nc.dram_tensor("out", [1, BPC], F32, kind="ExternalOutput")
    with tile.TileContext(nc) as tc:
        with ExitStack() as ctx:
            _kernel_body(ctx, tc, emb.ap(), winv.ap(), out.ap(), BPC, padb)
    nc.compile()
    _NC_CACHE[key] = nc
    return nc


def _pack(emb, labels):
    """Per-sample: smaller label group (padded to 512) then larger (padded to
    a uniform PADB, multiple of 16). Plain row order (xbar needs row-major).
    Returns (packed fp16 [B, 512+PADB, D], winv [B], count, PADB)."""
    npos = (labels == 1).sum(axis=1)
    nneg = N - npos
    big = int(np.maximum(npos, nneg).max())
    padb = max(528, -(-big // 16) * 16)
    rows = PADA + padb
    packed = np.zeros((B, rows, D), np.float16)
    for b in range(B):
        pos_idx = np.nonzero(labels[b] == 1)[0]
        neg_idx = np.nonzero(labels[b] == 0)[0]
        a_idx, b_idx = ((pos_idx, neg_idx) if len(pos_idx) <= len(neg_idx)
                        else (neg_idx, pos_idx))
        packed[b, :len(a_idx)] = emb[b, a_idx]
        packed[b, PADA:PADA + len(b_idx)] = emb[b, b_idx]
    valid = (npos > 0) & (nneg > 0)
    winv = (valid / np.maximum(nneg, 1)).astype(np.float32)
    count = int((npos * valid).sum())
    return packed, winv, count, padb


def kernel(embeddings: np.ndarray, labels: np.ndarray,
           _want_results=False, _trace=False) -> np.ndarray:
    emb = np.ascontiguousarray(embeddings, dtype=np.float32)
    lab = np.asarray(labels)
    assert emb.shape == (B, N, D) and lab.shape == (B, N)

    packed, winv, count, padb = _pack(emb, lab)
    nc = _build(padb)
    in_maps = [{"emb": packed[c * BPC:(c + 1) * BPC],
                "winv": winv[c * BPC:(c + 1) * BPC].reshape(1, BPC)}
               for c in range(NCORES)]
    res = bass_utils.run_bass_kernel_spmd(nc, in_maps,
                                          core_ids=list(range(NCORES)),
                                          trace=_trace)
    loss_sum = 0.0
    for c in range(NCORES):
        loss_sum += float(res.results[c]["out"].sum())
    ans = np.float32(loss_sum) / np.float32(max(count, 1))
    if _want_results:
        return np.float32(ans), res
    return np.float32(ans)


# revision 26
# speedup vs baseline: 1.0382x; 1.0382x over previous
"""nn_ContrastiveLoss Trainium2 kernel (8 NeuronCores, data-parallel over batch).

Contract: kernel(embeddings=[64,1024,128] f32, labels=[64,1024] int64) -> f32 scalar.

Sharding: batch dim B=64 split as 8 samples per core. Host packs each sample's
rows by label with the SMALLER group first (the "A side", <= 512 rows always,
zero-padded to 512) and the larger group second ("B side", zero-padded to
PADB), one uniform shape for all cores, cast to fp16 (halves HBM traffic).

Device pipeline per sample (measured-rate balanced):
  - DMA *xbar transpose* load: [rows,128] DRAM -> [128d, rows] SBUF fp16
    directly (no PE transposes, no diag build, no PSUM->SBUF copies)
  - squares on DVE (fp16 tensor_tensor, 2x mode)
  - row norms^2 via PE ones-matmuls (contraction over partitions), then tiny
    col-to-partition / rectangle DMA unflattens into a [128,20] layout where
    sqrt (ACT) and reciprocal (DVE) run on all lanes
  - B-side normalize: GpSimd partition_broadcast of rinvB (its only per-sample
    op type - GpSimd ucode swaps cost ~5us) + DVE fp16 multiply
  - A-side normalize is folded into the hinge via
      relu(rinvA_i*G' - th) = rinvA_i * relu(G' - th*rA_i):
    per-partition bias -th*rA_i in the hinge op, rinvA_i applied later to the
    accumulated per-row sums (one tiny batched multiply)
  - PE sim matmuls fp16 raw-A x normalized-B -> PSUM fp32
  - fused hinge+row-sum in ONE pass per chunk: ACT Relu(bias)+accum_out and
    DVE scalar_tensor_tensor (G+bias) max 0 with accum_out, exact (no
    correction term), split across both engines
  - epilogue: slots * rinvA, reduce, GpSimd cross-partition all-reduce,
    * host-provided valid/max(n_neg,1), DMA [1,8] losses
Host sums the 64 per-sample losses, divides by the label-derived count.
"""

import sys

if "/opt/trn_rl_repo" not in sys.path:
    sys.path.insert(0, "/opt/trn_rl_repo")

from contextlib import ExitStack

import numpy as np

import concourse.bass as bass
import concourse.bacc as bacc
import concourse.mybir as mybir
import concourse.tile as tile
from concourse import bass_isa, bass_utils

F32 = mybir.dt.float32
F16 = mybir.dt.float16
AF = mybir.ActivationFunctionType
ALU = mybir.AluOpType

P = 128
D = 128
N = 1024
B = 64
NCORES = 8
BPC = B // NCORES
PADA = 512          # lhsT side (min(npos,nneg) <= 512 always)
NA = PADA // P      # 4 A-chunks
THRESH = 0.5 - 0.35
EPS2 = 1e-6

# hinge engine per (sample, [wide0, wide1, tail]): wide = [128, 2*512] over
# two chunks' 512-wide sim parts; tail = [128, 4*64] over all four chunks'
# B-tail parts. Split ACT/DVE by measured rates (~9 ACT wides / 7 DVE wides).
_HINGE_PAT = [
    ["act", "dve", "dve"],
    ["dve", "act", "dve"],
    ["dve", "act", "dve"],
    ["act", "dve", "dve"],
] * (BPC // 4)


def _kernel_body(ctx, tc, emb_ap, winv_ap, out_ap, bpc, padb):
    nc = tc.nc
    rows = PADA + padb
    nbp = padb // 16          # B-side partitions in [*,16] nat layout (36)

    const_pool = ctx.enter_context(tc.tile_pool(name="const", bufs=1))
    xtpool = ctx.enter_context(tc.tile_pool(name="xt", bufs=3))
    sqpool = ctx.enter_context(tc.tile_pool(name="sq", bufs=2))
    nrmpool = ctx.enter_context(tc.tile_pool(name="nrm", bufs=2))
    ebpool = ctx.enter_context(tc.tile_pool(name="eb", bufs=2))
    acc_pool = ctx.enter_context(tc.tile_pool(name="acc", bufs=1))
    nsq_psum = ctx.enter_context(tc.tile_pool(name="nsqps", bufs=1, space="PSUM"))
    sim_psum = ctx.enter_context(tc.tile_pool(name="simps", bufs=2, space="PSUM"))
    simb_psum = ctx.enter_context(tc.tile_pool(name="simbps", bufs=1, space="PSUM"))

    eps2 = const_pool.tile([P, 1], F32)
    nc.gpsimd.memset(eps2[:], EPS2)
    zz16 = const_pool.tile([P, 1], F16)
    nc.gpsimd.memset(zz16[:], 0.0)
    onesw = const_pool.tile([P, 1], F16)
    nc.gpsimd.memset(onesw[:], 1.0)

    warm = const_pool.tile([P, 1], F32)
    nc.scalar.activation(warm[:], eps2[:], AF.Abs_reciprocal_sqrt, bias=eps2[:])

    # Pre-warm the xbar-transpose and partition_broadcast mechanisms: their
    # first use signals completion early (cold-start race seen on sample 0).
    wxb = const_pool.tile([P, 16], F16)
    nc.sync.dma_start_transpose(wxb[:], emb_ap[0][0:16, :])
    wbc = const_pool.tile([P, 16], F16)
    nc.gpsimd.partition_broadcast(wbc[:], wxb[0:1, :])
    wsq = const_pool.tile([P, 16], F16)
    nc.vector.tensor_mul(wsq[:], wxb[:], wbc[:])

    winv_sb = const_pool.tile([1, bpc], F32)
    nc.sync.dma_start(winv_sb[:], winv_ap[:])

    slots = acc_pool.tile([P, bpc, 3], F32)
    nc.gpsimd.memset(slots[:], 0.0)
    neg_thr = const_pool.tile([P, 1], F32)
    nc.gpsimd.memset(neg_thr[:], -THRESH)

    hsc_a = const_pool.tile([P, 1024], F16)
    hsc_d = const_pool.tile([P, 1024], F16)

    xt_t, eb_t = {}, {}

    def emit_load(b):
        xt = xtpool.tile([P, rows], F16, tag="xt")
        xt_t[b] = xt
        nc.sync.dma_start_transpose(xt[:], emb_ap[b])

    def emit_norm(b):
        xt = xt_t[b]
        esq = sqpool.tile([P, rows], F16, tag="esq")
        nc.vector.tensor_mul(esq[:], xt[:], xt[:])

        # norms^2 via ones-matmuls into one flat PSUM tile; ONE ACT
        # Abs_reciprocal_sqrt = PSUM exit + rsqrt fused, single rfl writer
        # (keeps the GpSimd broadcast's semaphore count down)
        ps = nsq_psum.tile([1, rows], F32, tag="nsq")
        for c0 in range(0, rows, 512):
            w = min(512, rows - c0)
            nc.tensor.matmul(ps[0:1, c0:c0 + w], lhsT=onesw[:],
                             rhs=esq[:, c0:c0 + w], start=True, stop=True)
        rfl = nrmpool.tile([1, rows], F16, tag="rfl")
        nc.scalar.activation(rfl[:], ps[:], AF.Abs_reciprocal_sqrt,
                             bias=eps2[0:1, :])
        rbc = nrmpool.tile([P, rows], F16, tag="rbc")
        nc.gpsimd.partition_broadcast(rbc[:], rfl[0:1, :])
        eh = ebpool.tile([P, rows], F16, tag="eh")
        eb_t[b] = eh
        nc.vector.tensor_mul(eh[:], xt[:], rbc[:])

    tailw = padb - 512

    def hinge(which, src, shape, slot):
        if which == "act":
            nc.scalar.activation(hsc_a[:, 0:shape], src, AF.Relu,
                                 bias=neg_thr[:], accum_out=slot)
        else:
            nc.vector.scalar_tensor_tensor(
                hsc_d[:, 0:shape], src, THRESH,
                zz16[:].broadcast_to(src.shape),
                ALU.subtract, ALU.max, accum_out=slot)

    def emit_sims(b):
        eh = eb_t[b]
        # B-tail parts of all four chunks share one 1-bank tile, hinged once
        # (emitted before the last wide hinge so the next sample's tail
        # matmuls aren't gated on this sample's final hinge)
        sim_b = simb_psum.tile([P, NA, tailw], F32, tag="simb")
        sims_a = []
        for half in range(2):
            sim_a = sim_psum.tile([P, 2, 512], F32, tag="sima")
            sims_a.append(sim_a)
            for q in range(2):
                mt = half * 2 + q
                lhs = eh[:, mt * P:(mt + 1) * P]
                nc.tensor.matmul(sim_a[:, q, :], lhsT=lhs,
                                 rhs=eh[:, PADA:PADA + 512],
                                 start=True, stop=True)
                nc.tensor.matmul(sim_b[:, mt, :], lhsT=lhs,
                                 rhs=eh[:, PADA + 512:rows],
                                 start=True, stop=True)
            if half == 0:
                hinge(_HINGE_PAT[b][0], sim_a[:], 1024,
                      slots[:, b, 0:1])
        hinge(_HINGE_PAT[b][2], sim_b[:], NA * tailw, slots[:, b, 2:3])
        hinge(_HINGE_PAT[b][1], sims_a[1][:], 1024, slots[:, b, 1:2])
        del xt_t[b], eb_t[b]

    emit_load(0)
    emit_load(1)
    emit_norm(0)
    for b in range(bpc):
        if b + 2 < bpc:
            emit_load(b + 2)
        if b + 1 < bpc:
            emit_norm(b + 1)
        emit_sims(b)

    red = acc_pool.tile([P, bpc], F32)
    nc.vector.tensor_reduce(red[:], slots[:], axis=mybir.AxisListType.X,
                            op=ALU.add)
    redr = acc_pool.tile([P, bpc], F32)
    nc.gpsimd.partition_all_reduce(redr[:], red[:], channels=P,
                                   reduce_op=bass_isa.ReduceOp.add)
    loss = acc_pool.tile([1, bpc], F32)
    nc.vector.tensor_mul(loss[:], redr[0:1, :], winv_sb[:])
    nc.sync.dma_start(out_ap[0:1, :], loss[0:1, :])


_NC_CACHE = {}


def _build(padb):
    key = (BPC, NCORES, padb)
    if key in _NC_CACHE:
        return _NC_CACHE[key]
    nc = bacc.Bacc("TRN2", target_bir_lowering=False, debug=False,
                   num_devices=NCORES)
    emb = nc.dram_tensor("emb", [BPC, PADA + padb, D], F16,
                         kind="ExternalInput")
    winv = nc.dram_tensor("winv", [1, BPC], F32, kind="ExternalInput")
    out = nc.dram_tensor("out", [1, BPC], F32, kind="ExternalOutput")
    with tile.TileContext(nc) as tc:
        with ExitStack() as ctx:
            _kernel_body(ctx, tc, emb.ap(), winv.ap(), out.ap(), BPC, padb)
    nc.compile()
    _NC_CACHE[key] = nc
    return nc


def _pack(emb, labels):
    """Per-sample: smaller label group (padded to 512) then larger (padded to
    a uniform PADB, multiple of 16). Plain row order (xbar needs row-major).
    Returns (packed fp16 [B, 512+PADB, D], winv [B], count, PADB)."""
    npos = (labels == 1).sum(axis=1)
    nneg = N - npos
    big = int(np.maximum(npos, nneg).max())
    padb = max(528, -(-big // 16) * 16)
    rows = PADA + padb
    packed = np.zeros((B, rows, D), np.float16)
    for b in range(B):
        pos_idx = np.nonzero(labels[b] == 1)[0]
        neg_idx = np.nonzero(labels[b] == 0)[0]
        a_idx, b_idx = ((pos_idx, neg_idx) if len(pos_idx) <= len(neg_idx)
                        else (neg_idx, pos_idx))
        packed[b, :len(a_idx)] = emb[b, a_idx]
        packed[b, PADA:PADA + len(b_idx)] = emb[b, b_idx]
    valid = (npos > 0) & (nneg > 0)
    winv = (valid / np.maximum(nneg, 1)).astype(np.float32)
    count = int((npos * valid).sum())
    return packed, winv, count, padb


def kernel(embeddings: np.ndarray, labels: np.ndarray,
           _want_results=False, _trace=False) -> np.ndarray:
    emb = np.ascontiguousarray(embeddings, dtype=np.float32)
    lab = np.asarray(labels)
    assert emb.shape == (B, N, D) and lab.shape == (B, N)

    packed, winv, count, padb = _pack(emb, lab)
    nc = _build(padb)
    in_maps = [{"emb": packed[c * BPC:(c + 1) * BPC],
                "winv": winv[c * BPC:(c + 1) * BPC].reshape(1, BPC)}
               for c in range(NCORES)]
    res = bass_utils.run_bass_kernel_spmd(nc, in_maps,
                                          core_ids=list(range(NCORES)),
                                          trace=_trace)
    loss_sum = 0.0
    for c in range(NCORES):
        loss_sum += float(res.results[c]["out"].sum())
    ans = np.float32(loss_sum) / np.float32(max(count, 1))
    if _want_results:
        return np.float32(ans), res
    return np.float32(ans)


# revision 27
# speedup vs baseline: 1.2137x; 1.1690x over previous
"""nn_ContrastiveLoss Trainium2 kernel (8 NeuronCores, data-parallel over batch).

Contract: kernel(embeddings=[64,1024,128] f32, labels=[64,1024] int64) -> f32 scalar.

Sharding: batch dim B=64 split as 8 samples per core. Host packs each sample's
rows by label with the SMALLER group first (the "A side", <= 512 rows always,
zero-padded to 512) and the larger group second ("B side", zero-padded to
PADB), one uniform shape for all cores, cast to fp16 (halves HBM traffic).

Device pipeline per sample (measured-rate balanced):
  - DMA *xbar transpose* load: [rows,128] DRAM -> [128d, rows] SBUF fp16
    directly (no PE transposes, no diag build, no PSUM->SBUF copies)
  - squares on DVE (fp16 tensor_tensor, 2x mode)
  - row norms^2 via PE ones-matmuls (contraction over partitions), then tiny
    col-to-partition / rectangle DMA unflattens into a [128,20] layout where
    sqrt (ACT) and reciprocal (DVE) run on all lanes
  - B-side normalize: GpSimd partition_broadcast of rinvB (its only per-sample
    op type - GpSimd ucode swaps cost ~5us) + DVE fp16 multiply
  - A-side normalize is folded into the hinge via
      relu(rinvA_i*G' - th) = rinvA_i * relu(G' - th*rA_i):
    per-partition bias -th*rA_i in the hinge op, rinvA_i applied later to the
    accumulated per-row sums (one tiny batched multiply)
  - PE sim matmuls fp16 raw-A x normalized-B -> PSUM fp32
  - fused hinge+row-sum in ONE pass per chunk: ACT Relu(bias)+accum_out and
    DVE scalar_tensor_tensor (G+bias) max 0 with accum_out, exact (no
    correction term), split across both engines
  - epilogue: slots * rinvA, reduce, GpSimd cross-partition all-reduce,
    * host-provided valid/max(n_neg,1), DMA [1,8] losses
Host sums the 64 per-sample losses, divides by the label-derived count.
"""

import sys

if "/opt/trn_rl_repo" not in sys.path:
    sys.path.insert(0, "/opt/trn_rl_repo")

from contextlib import ExitStack

import numpy as np

import concourse.bass as bass
import concourse.bacc as bacc
import concourse.mybir as mybir
import concourse.tile as tile
from concourse import bass_isa, bass_utils

F32 = mybir.dt.float32
F16 = mybir.dt.float16
AF = mybir.ActivationFunctionType
ALU = mybir.AluOpType

P = 128
D = 128
N = 1024
B = 64
NCORES = 8
BPC = B // NCORES
PADA = 512          # lhsT side (min(npos,nneg) <= 512 always)
NA = PADA // P      # 4 A-chunks
THRESH = 0.5 - 0.35
EPS2 = 1e-6

# hinge engine per (sample, [wide0, wide1, tail]): wide = [128, 2*512] over
# two chunks' 512-wide sim parts; tail = [128, 4*64] over all four chunks'
# B-tail parts. Split ACT/DVE by measured rates (~9 ACT wides / 7 DVE wides).
_HINGE_PAT = [
    ["act", "dve", "dve"],
    ["dve", "act", "dve"],
    ["act", "dve", "dve"],
    ["act", "dve", "dve"],
] * (BPC // 4)


def _kernel_body(ctx, tc, emb_ap, winv_ap, out_ap, bpc, padb):
    nc = tc.nc
    rows = PADA + padb
    nbp = padb // 16          # B-side partitions in [*,16] nat layout (36)

    const_pool = ctx.enter_context(tc.tile_pool(name="const", bufs=1))
    xtpool = ctx.enter_context(tc.tile_pool(name="xt", bufs=3))
    sqpool = ctx.enter_context(tc.tile_pool(name="sq", bufs=2))
    nrmpool = ctx.enter_context(tc.tile_pool(name="nrm", bufs=2))
    ebpool = ctx.enter_context(tc.tile_pool(name="eb", bufs=2))
    acc_pool = ctx.enter_context(tc.tile_pool(name="acc", bufs=1))
    nsq_psum = ctx.enter_context(tc.tile_pool(name="nsqps", bufs=1, space="PSUM"))
    sim_psum = ctx.enter_context(tc.tile_pool(name="simps", bufs=2, space="PSUM"))
    simb_psum = ctx.enter_context(tc.tile_pool(name="simbps", bufs=1, space="PSUM"))

    eps2 = const_pool.tile([P, 1], F32)
    nc.gpsimd.memset(eps2[:], EPS2)
    zz16 = const_pool.tile([P, 1], F16)
    nc.gpsimd.memset(zz16[:], 0.0)
    onesw = const_pool.tile([P, 1], F16)
    nc.gpsimd.memset(onesw[:], 1.0)

    warm = const_pool.tile([P, 1], F32)
    nc.scalar.activation(warm[:], eps2[:], AF.Abs_reciprocal_sqrt, bias=eps2[:])

    # Pre-warm the xbar-transpose and partition_broadcast mechanisms: their
    # first use signals completion early (cold-start race seen on sample 0).
    wxb = const_pool.tile([P, 16], F16)
    nc.sync.dma_start_transpose(wxb[:], emb_ap[0][0:16, :])
    wbc = const_pool.tile([P, 16], F16)
    nc.gpsimd.partition_broadcast(wbc[:], wxb[0:1, :])
    wsq = const_pool.tile([P, 16], F16)
    nc.vector.tensor_mul(wsq[:], wxb[:], wbc[:])

    winv_sb = const_pool.tile([1, bpc], F32)
    nc.sync.dma_start(winv_sb[:], winv_ap[:])

    slots = acc_pool.tile([P, bpc, 3], F32)
    nc.gpsimd.memset(slots[:], 0.0)
    neg_thr = const_pool.tile([P, 1], F32)
    nc.gpsimd.memset(neg_thr[:], -THRESH)

    hsc_a = const_pool.tile([P, 1024], F16)
    hsc_d = const_pool.tile([P, 1024], F16)

    xt_t, eb_t = {}, {}

    def emit_load(b):
        xt = xtpool.tile([P, rows], F16, tag="xt")
        xt_t[b] = xt
        nc.sync.dma_start_transpose(xt[:], emb_ap[b])

    def emit_norm(b):
        xt = xt_t[b]
        esq = sqpool.tile([P, rows], F16, tag="esq")
        nc.vector.tensor_mul(esq[:], xt[:], xt[:])

        # norms^2 via ones-matmuls into one flat PSUM tile; ONE ACT
        # Abs_reciprocal_sqrt = PSUM exit + rsqrt fused, single rfl writer
        # (keeps the GpSimd broadcast's semaphore count down)
        ps = nsq_psum.tile([1, rows], F32, tag="nsq")
        for c0 in range(0, rows, 512):
            w = min(512, rows - c0)
            nc.tensor.matmul(ps[0:1, c0:c0 + w], lhsT=onesw[:],
                             rhs=esq[:, c0:c0 + w], start=True, stop=True)
        rfl = nrmpool.tile([1, rows], F16, tag="rfl")
        nc.scalar.activation(rfl[:], ps[:], AF.Abs_reciprocal_sqrt,
                             bias=eps2[0:1, :])
        rbc = nrmpool.tile([P, rows], F16, tag="rbc")
        nc.gpsimd.partition_broadcast(rbc[:], rfl[0:1, :])
        eh = ebpool.tile([P, rows], F16, tag="eh")
        eb_t[b] = eh
        nc.vector.tensor_mul(eh[:], xt[:], rbc[:])

    tailw = padb - 512

    def hinge(which, src, shape, slot):
        if which == "act":
            nc.scalar.activation(hsc_a[:, 0:shape], src, AF.Relu,
                                 bias=neg_thr[:], accum_out=slot)
        else:
            nc.vector.scalar_tensor_tensor(
                hsc_d[:, 0:shape], src, THRESH,
                zz16[:].broadcast_to(src.shape),
                ALU.subtract, ALU.max, accum_out=slot)

    def emit_sims(b):
        eh = eb_t[b]
        # B-tail parts of all four chunks share one 1-bank tile, hinged once
        # (emitted before the last wide hinge so the next sample's tail
        # matmuls aren't gated on this sample's final hinge)
        sim_b = simb_psum.tile([P, NA, tailw], F32, tag="simb")
        sims_a = []
        for half in range(2):
            sim_a = sim_psum.tile([P, 2, 512], F32, tag="sima")
            sims_a.append(sim_a)
            for q in range(2):
                mt = half * 2 + q
                lhs = eh[:, mt * P:(mt + 1) * P]
                nc.tensor.matmul(sim_a[:, q, :], lhsT=lhs,
                                 rhs=eh[:, PADA:PADA + 512],
                                 start=True, stop=True)
                nc.tensor.matmul(sim_b[:, mt, :], lhsT=lhs,
                                 rhs=eh[:, PADA + 512:rows],
                                 start=True, stop=True)
            if half == 0:
                hinge(_HINGE_PAT[b][0], sim_a[:], 1024,
                      slots[:, b, 0:1])
        hinge(_HINGE_PAT[b][2], sim_b[:], NA * tailw, slots[:, b, 2:3])
        hinge(_HINGE_PAT[b][1], sims_a[1][:], 1024, slots[:, b, 1:2])
        del xt_t[b], eb_t[b]

    emit_load(0)
    emit_load(1)
    emit_norm(0)
    for b in range(bpc):
        if b + 2 < bpc:
            emit_load(b + 2)
        if b + 1 < bpc:
            emit_norm(b + 1)
        emit_sims(b)

    red = acc_pool.tile([P, bpc], F32)
    nc.vector.tensor_reduce(red[:], slots[:], axis=mybir.AxisListType.X,
                            op=ALU.add)
    redr = acc_pool.tile([P, bpc], F32)
    nc.gpsimd.partition_all_reduce(redr[:], red[:], channels=P,
                                   reduce_op=bass_isa.ReduceOp.add)
    loss = acc_pool.tile([1, bpc], F32)
    nc.vector.tensor_mul(loss[:], redr[0:1, :], winv_sb[:])
    nc.sync.dma_start(out_ap[0:1, :], loss[0:1, :])


_NC_CACHE = {}


def _build(padb):
    key = (BPC, NCORES, padb)
    if key in _NC_CACHE:
        return _NC_CACHE[key]
    nc = bacc.Bacc("TRN2", target_bir_lowering=False, debug=False,
                   num_devices=NCORES)
    emb = nc.dram_tensor("emb", [BPC, PADA + padb, D], F16,
                         kind="ExternalInput")
    winv = nc.dram_tensor("winv", [1, BPC], F32, kind="ExternalInput")
    out = nc.dram_tensor("out", [1, BPC], F32, kind="ExternalOutput")
    with tile.TileContext(nc) as tc:
        with ExitStack() as ctx:
            _kernel_body(ctx, tc, emb.ap(), winv.ap(), out.ap(), BPC, padb)
    nc.compile()
    _NC_CACHE[key] = nc
    return nc


def _pack(emb, labels):
    """Per-sample: smaller label group (padded to 512) then larger (padded to
    a uniform PADB, multiple of 16). Plain row order (xbar needs row-major).
    Returns (packed fp16 [B, 512+PADB, D], winv [B], count, PADB)."""
    npos = (labels == 1).sum(axis=1)
    nneg = N - npos
    big = int(np.maximum(npos, nneg).max())
    padb = max(528, -(-big // 16) * 16)
    rows = PADA + padb
    packed = np.zeros((B, rows, D), np.float16)
    for b in range(B):
        pos_idx = np.nonzero(labels[b] == 1)[0]
        neg_idx = np.nonzero(labels[b] == 0)[0]
        a_idx, b_idx = ((pos_idx, neg_idx) if len(pos_idx) <= len(neg_idx)
                        else (neg_idx, pos_idx))
        packed[b, :len(a_idx)] = emb[b, a_idx]
        packed[b, PADA:PADA + len(b_idx)] = emb[b, b_idx]
    valid = (npos > 0) & (nneg > 0)
    winv = (valid / np.maximum(nneg, 1)).astype(np.float32)
    count = int((npos * valid).sum())
    return packed, winv, count, padb


def kernel(embeddings: np.ndarray, labels: np.ndarray,
           _want_results=False, _trace=False) -> np.ndarray:
    emb = np.ascontiguousarray(embeddings, dtype=np.float32)
    lab = np.asarray(labels)
    assert emb.shape == (B, N, D) and lab.shape == (B, N)

    packed, winv, count, padb = _pack(emb, lab)
    nc = _build(padb)
    in_maps = [{"emb": packed[c * BPC:(c + 1) * BPC],
                "winv": winv[c * BPC:(c + 1) * BPC].reshape(1, BPC)}
               for c in range(NCORES)]
    res = bass_utils.run_bass_kernel_spmd(nc, in_maps,
                                          core_ids=list(range(NCORES)),
                                          trace=_trace)
    loss_sum = 0.0
    for c in range(NCORES):
        loss_sum += float(res.results[c]["out"].sum())
    ans = np.float32(loss_sum) / np.float32(max(count, 1))
    if _want_results:
        return np.float32(ans), res
    return np.float32(ans)
